# revision 65
# baseline (speedup 1.0000x reference)
"""Trainium2 Bass kernel for nn_CrossAttention_47502338294587.

Math: the reference cross-attention has a single KV position broadcast over
all T query positions.  Softmax over a row of identical logits is uniform,
so attention output == v for every query, and the whole module collapses to

    out[b, t, :] = (visual_features[b] @ Wv + bv) @ Wp + bp      (for all t)

independent of x / Wq / Wk.  The device computes the two projections and
broadcasts the per-batch row over the T axis; the host only does input
layout prep (incl. bf16 quantization of the weights) and shard re-assembly.

Sharding: tensor-parallel over the output channel dim C — core i computes
and writes out[:, :, i*128:(i+1)*128].

Per-core structure (matmuls bf16, PSUM accumulation fp32):
  warm:  self-sufficient matmuls on memset tiles ramp the PE clock
         (p-state) before real work; vf^T transposes run on the warm PE
  mm1:   vv = [1|vf] @ [bv; Wv]   moving Wv 2-chunk pairs (4KB DMA
         lines on the two HWDGE queues; SWDGE/gpsimd is ~2x slower and
         only carries late-needed tensors)
  tr:    vv^T chunks via PE transpose into ONE psum tile, one copy out
  mm2:   row = [1|vv] @ [bp; Wp[:,ci]]  (bias as rank-1 term)
  bcast: rhs4 = rep4(row)*sel (DVE), bc = ones^T @ rhs4 (one matmul),
         psum -> SBUF bf16 halves (DVE + Act), then 2 replicated
         out-DMAs (step-0 over the 8 t-chunks) on the two HWDGE queues;
         output is bf16, host widens to fp32 (exact)

Measured phase budget (HW, per core): ~7us fixed preamble (engine IRAM
loads, barriers), ~9us in-DMA (chip HBM-limited: 8 cores x 2.3MB),
~4us compute tail, ~4.5us out-DMA (8 x 1MB bf16), ~6us fixed semaphore
teardown.  44.2us baseline -> ~32us.
"""

import os
import sys

import numpy as np
import ml_dtypes

for _p in ("/opt/trn_rl_repo",):
    if _p not in sys.path and os.path.isdir(_p):
        sys.path.insert(0, _p)

B, T, C = 4, 1024, 1024
N_CORES = 8
CSH = C // N_CORES  # 128, C-shard per core
KC = C // 128  # 8 contraction chunks

BF16 = ml_dtypes.bfloat16

_BUILT = None


def build_nc():
    """Build + compile the Bass program (one NeuronCore's SPMD body)."""
    import concourse.bass as bass
    import concourse.mybir as mybir
    import concourse.tile as tile
    from concourse import bacc
    from concourse.bass import ts

    f32 = mybir.dt.float32
    bf16 = mybir.dt.bfloat16
    nc = bacc.Bacc("TRN2", target_bir_lowering=False, debug=False)

    # host pre-packs into the exact SBUF layouts (layout + bf16 quantization)
    wv_b = nc.dram_tensor("wv_b", [128, KC * C], bf16, kind="ExternalInput")
    wp_b = nc.dram_tensor("wp_b", [128, KC * CSH], bf16, kind="ExternalInput")
    # vf row b || eye row b: one DMA, 2056-byte lines
    vfeye_b = nc.dram_tensor("vfeye_b", [B, C + B], bf16, kind="ExternalInput")
    bvbp_b = nc.dram_tensor("bvbp_b", [1, C + CSH], bf16, kind="ExternalInput")
    sel_b = nc.dram_tensor("sel_b", [B, B * CSH], bf16, kind="ExternalInput")
    # out[t, b, c_local] in bf16 (host widens to fp32 -- exact cast);
    # host re-assembles full[b, t, ci] = out[t, b, :]
    out = nc.dram_tensor("out", [T, B, CSH], bf16, kind="ExternalOutput")

    with tile.TileContext(nc) as tc:
        with tc.tile_pool(name="sb", bufs=1) as sb:
            # ---- SBUF tiles -------------------------------------------------
            wv2_t = [
                sb.tile([128, 2 * C], bf16, name=f"wv{j}", tag=f"wv{j}")
                for j in range(KC // 2)
            ]
            warm_t = sb.tile([1, 512], bf16, tag="warm")
            wp_t = sb.tile([128, KC, CSH], bf16, tag="wp_t")
            vfeye_t = sb.tile([B, C + B], bf16, tag="vfeye")
            vft_t = sb.tile([128, KC * B], bf16, tag="vft")
            bvbp_t = sb.tile([1, C + CSH], bf16, tag="bvbp")
            sel_t = sb.tile([B, B * CSH], bf16, tag="sel")
            ones_t = sb.tile([1, B], bf16, tag="ones")
            ones_bc = sb.tile([B, 128], bf16, tag="ones_bc")
            vv_sb = sb.tile([B, C], bf16, tag="vv_sb")
            vvt_t = sb.tile([128, KC * B], bf16, tag="vvt")
            rhs4_t = sb.tile([B, B * CSH], bf16, tag="rhs4")
            bc_sb = sb.tile([128, B * CSH], bf16, tag="bc")

            nc.gpsimd.memset(ones_t[:], 1.0)
            nc.gpsimd.memset(ones_bc[:], 1.0)
            nc.gpsimd.memset(warm_t[:], 0.5)

            # ---- DMA in: three-way balanced split in k-need order -----------
            # sync/scalar (~150 GB/s each) carry pairs 01/23 + singles 4/5;
            # the late pair 67 rides the slow gpsimd queue (needed last)
            nc.sync.dma_start(vfeye_t[:], vfeye_b[:, :])
            nc.sync.dma_start(wv2_t[0][:], wv_b[:, 0 : 2 * C])
            nc.sync.dma_start(wv2_t[2][:, 0:C], wv_b[:, 4 * C : 5 * C])
            nc.scalar.dma_start(bvbp_t[:], bvbp_b[:, :])
            nc.scalar.dma_start(wv2_t[1][:], wv_b[:, 2 * C : 4 * C])
            nc.scalar.dma_start(wv2_t[2][:, C : 2 * C], wv_b[:, 5 * C : 6 * C])
            nc.gpsimd.dma_start(wv2_t[3][:], wv_b[:, 6 * C : 8 * C])
            nc.gpsimd.dma_start(sel_t[:], sel_b[:, :])
            nc.gpsimd.dma_start(wp_t[:], wp_b.rearrange("p (k c) -> p k c", c=CSH))

            with (
                tc.tile_pool(name="pv", bufs=1, space="PSUM") as pv,
                tc.tile_pool(name="pf", bufs=1, space="PSUM") as pf,
                tc.tile_pool(name="pw", bufs=1, space="PSUM") as pw,
            ):
                psum_vv = [
                    pv.tile([B, 512], mybir.dt.float32, name=f"pvv{h}", tag=f"pvv{h}")
                    for h in range(2)
                ]
                # self-sufficient warm-up matmuls (memset inputs only -- no
                # DMA waits can be hoisted onto them): ramp the PE clock
                psum_warm = pw.tile([B, 512], mybir.dt.float32, tag="pwm")
                for _ in range(5):
                    nc.tensor.matmul(
                        psum_warm[:],
                        ones_t[0:1, :],
                        warm_t[0:1, :],
                        start=True,
                        stop=True,
                        skip_group_check=True,
                    )

                # ---- vf^T chunks via PE transpose (on the warming PE) -------
                psum_vft = pf.tile([128, KC * B], bf16, tag="pvf")
                for k in range(KC):
                    nc.tensor.transpose(
                        psum_vft[:, ts(k, B)],
                        vfeye_t[0:B, ts(k, 128)],
                        vfeye_t[0:B, C : C + B],
                    )
                nc.vector.tensor_copy(vft_t[:], psum_vft[:])

                # rank-1 bias terms: psum_vv[h] = 1^T bv_half
                for h in range(2):
                    nc.tensor.matmul(
                        psum_vv[h][:],
                        ones_t[0:1, :],
                        bvbp_t[0:1, ts(h, 512)],
                        start=True,
                        stop=False,
                        skip_group_check=True,
                    )

                # ---- mm1: vv[b, n] = bv[n] + sum_k vf[b, k] Wv[k, n] --------
                for k in range(KC):
                    for h in range(2):
                        nc.tensor.matmul(
                            psum_vv[h][:],
                            vft_t[:, ts(k, B)],
                            wv2_t[k // 2][:, ts(2 * (k % 2) + h, 512)],
                            start=False,
                            stop=(k == KC - 1),
                            skip_group_check=True,
                        )
                # psum -> SBUF bf16 cast, split across DVE + Act engines
                nc.vector.tensor_copy(vv_sb[0:B, 0:512], psum_vv[0][:])
                nc.scalar.activation(
                    vv_sb[0:B, 512:1024],
                    psum_vv[1][:],
                    mybir.ActivationFunctionType.Copy,
                )

            # ---- transpose vv -> vv^T into one psum tile, one copy out ------
            with tc.tile_pool(name="pt", bufs=1, space="PSUM") as pt:
                psum_vvt = pt.tile([128, KC * B], bf16, tag="pvt")
                for k in range(KC):
                    nc.tensor.transpose(
                        psum_vvt[:, ts(k, B)],
                        vv_sb[0:B, ts(k, 128)],
                        vfeye_t[0:B, C : C + B],
                    )
                nc.vector.tensor_copy(vvt_t[:], psum_vvt[:])

            # ---- mm2: row = bp + vv @ Wp[:,ci] ------------------------------
            with (
                tc.tile_pool(name="pr", bufs=1, space="PSUM") as pr,
                tc.tile_pool(name="pb", bufs=1, space="PSUM") as pb,
            ):
                psum_row = pr.tile([B, CSH], mybir.dt.float32, tag="pr")
                nc.tensor.matmul(
                    psum_row[:],
                    ones_t[0:1, :],
                    bvbp_t[0:1, C : C + CSH],
                    start=True,
                    stop=False,
                )
                for k in range(KC):
                    nc.tensor.matmul(
                        psum_row[:],
                        vvt_t[:, ts(k, B)],
                        wp_t[:, k, :],
                        start=False,
                        stop=(k == KC - 1),
                    )

                # ---- T-broadcast: bc[t, (q,c)] = row[q, c] ------------------
                pra = psum_row[:]
                prep = bass.AP(
                    pra.tensor, pra.offset, [list(pra.ap[0]), [0, B], list(pra.ap[1])]
                )
                nc.vector.tensor_mul(
                    rhs4_t[:].rearrange("p (q f) -> p q f", q=B),
                    prep,
                    sel_t[:].rearrange("p (q f) -> p q f", q=B),
                )
                psum_bc = pb.tile([128, B * CSH], mybir.dt.float32, tag="pb")
                nc.tensor.matmul(
                    psum_bc[:], ones_bc[0:B, :], rhs4_t[0:B, :], start=True, stop=True
                )
                # psum -> SBUF bf16 halves (DVE + Act in parallel), then the
                # replicated out-DMAs (step-0 over the 8 t-chunks)
                half = B * CSH // 2
                out_v = out.rearrange("(q p) b c -> p q (b c)", p=128)
                nc.vector.tensor_copy(bc_sb[:, 0:half], psum_bc[:, 0:half])
                nc.scalar.activation(
                    bc_sb[:, half:],
                    psum_bc[:, half:],
                    mybir.ActivationFunctionType.Copy,
                )
                for i, deng in ((0, nc.sync), (1, nc.scalar)):
                    ap = bc_sb[:, i * half : (i + 1) * half]
                    rep = bass.AP(
                        ap.tensor, ap.offset, [list(ap.ap[0]), [0, KC], list(ap.ap[1])]
                    )
                    deng.dma_start(out_v[:, :, i * half : (i + 1) * half], rep)

    nc.compile()
    return nc


def _get_built():
    global _BUILT
    if _BUILT is None:
        _BUILT = build_nc()
    return _BUILT


def make_in_maps(inputs):
    vf = np.asarray(inputs["visual_features"], np.float32)
    wv = np.asarray(inputs["Wv"], np.float32)
    wp = np.asarray(inputs["Wp"], np.float32)
    bv = np.asarray(inputs["bv"], np.float32)
    bp = np.asarray(inputs["bp"], np.float32)
    # wv_b[p, k*C + n] = Wv[k*128 + p, n]
    wv_b = np.ascontiguousarray(
        wv.reshape(KC, 128, C).transpose(1, 0, 2).reshape(128, KC * C)
    ).astype(BF16)
    vfeye_b = np.concatenate([vf, np.eye(B, dtype=np.float32)], axis=1).astype(BF16)
    sel_b = np.zeros((B, B * CSH), np.float32)
    for b in range(B):
        sel_b[b, b * CSH : (b + 1) * CSH] = 1.0
    sel_b = sel_b.astype(BF16)
    maps = []
    for i in range(N_CORES):
        ci = slice(i * CSH, (i + 1) * CSH)
        # wp_b[p, k*CSH + c] = Wp[k*128 + p, ci_c]
        wp_b = np.ascontiguousarray(
            wp[:, ci].reshape(KC, 128, CSH).transpose(1, 0, 2).reshape(128, KC * CSH)
        ).astype(BF16)
        bvbp_b = np.concatenate([bv, bp[ci]]).reshape(1, C + CSH).astype(BF16)
        maps.append(
            {
                "wv_b": wv_b,
                "wp_b": wp_b,
                "vfeye_b": vfeye_b,
                "bvbp_b": bvbp_b,
                "sel_b": sel_b,
            }
        )
    return maps


def run(inputs, trace=False, **kw):
    from concourse.bass_utils import run_bass_kernel_spmd

    nc = _get_built()
    res = run_bass_kernel_spmd(
        nc,
        make_in_maps(inputs),
        core_ids=list(range(N_CORES)),
        trace=trace,
        **kw,
    )
    full = np.empty((B, T, C), np.float32)
    for i, r in enumerate(res.results):
        full[:, :, i * CSH : (i + 1) * CSH] = (
            r["out"].astype(np.float32).transpose(1, 0, 2)
        )
    return full, res


def kernel(**inputs) -> np.ndarray:
    full, _ = run(inputs, trace=False)
    return full


# revision 66
# speedup vs baseline: 1.1528x; 1.1528x over previous
"""Trainium2 Bass kernel for nn_CrossAttention_47502338294587.

Math: the reference cross-attention has a single KV position broadcast over
all T query positions.  Softmax over a row of identical logits is uniform,
so attention output == v for every query, and the whole module collapses to

    out[b, t, :] = (visual_features[b] @ Wv + bv) @ Wp + bp      (for all t)

independent of x / Wq / Wk.  The device computes the two projections and
broadcasts the per-batch row over the T axis; the host only does input
layout prep (incl. bf16 quantization of the weights) and shard re-assembly.

Sharding: tensor-parallel over the output channel dim C — core i computes
and writes out[:, :, i*128:(i+1)*128].

Per-core structure (matmuls bf16, PSUM accumulation fp32):
  warm:  self-sufficient matmuls on memset tiles ramp the PE clock
         (p-state) before real work; vf^T transposes run on the warm PE
  mm1:   vv = [1|vf] @ [bv; Wv]   moving Wv 2-chunk pairs (4KB DMA
         lines on the two HWDGE queues; SWDGE/gpsimd is ~2x slower and
         only carries late-needed tensors)
  tr:    vv^T chunks via PE transpose into ONE psum tile, one copy out
  mm2:   row = [1|vv] @ [bp; Wp[:,ci]]  (bias as rank-1 term)
  bcast: rhs4 = rep4(row)*sel (DVE), bc = ones^T @ rhs4 (one matmul),
         psum -> SBUF bf16 halves (DVE + Act), then 2 replicated
         out-DMAs (step-0 over the 8 t-chunks) on the two HWDGE queues;
         output is bf16, host widens to fp32 (exact)

Measured phase budget (HW, per core): ~7us fixed preamble (engine IRAM
loads, barriers), ~9us in-DMA (chip HBM-limited: 8 cores x 2.3MB),
~4us compute tail, ~4.5us out-DMA (8 x 1MB bf16), ~6us fixed semaphore
teardown.  44.2us baseline -> ~32us.
"""

import os
import sys

import numpy as np
import ml_dtypes

for _p in ("/opt/trn_rl_repo",):
    if _p not in sys.path and os.path.isdir(_p):
        sys.path.insert(0, _p)

B, T, C = 4, 1024, 1024
N_CORES = 8
CSH = C // N_CORES  # 128, C-shard per core
KC = C // 128  # 8 contraction chunks

BF16 = ml_dtypes.bfloat16

_BUILT = None


def build_nc():
    """Build + compile the Bass program (one NeuronCore's SPMD body)."""
    import concourse.bass as bass
    import concourse.mybir as mybir
    import concourse.tile as tile
    from concourse import bacc
    from concourse.bass import ts

    f32 = mybir.dt.float32
    bf16 = mybir.dt.bfloat16
    nc = bacc.Bacc("TRN2", target_bir_lowering=False, debug=False)

    # host pre-packs into the exact SBUF layouts (layout + bf16 quantization)
    wv_b = nc.dram_tensor("wv_b", [128, KC * C], bf16, kind="ExternalInput")
    wp_b = nc.dram_tensor("wp_b", [128, KC * CSH], bf16, kind="ExternalInput")
    # vf row b || eye row b: one DMA, 2056-byte lines
    vfeye_b = nc.dram_tensor("vfeye_b", [B, C + B], bf16, kind="ExternalInput")
    bvbp_b = nc.dram_tensor("bvbp_b", [1, C + CSH], bf16, kind="ExternalInput")
    sel_b = nc.dram_tensor("sel_b", [B, B * CSH], bf16, kind="ExternalInput")
    # out[t, b, c_local] in bf16 (host widens to fp32 -- exact cast);
    # host re-assembles full[b, t, ci] = out[t, b, :]
    out = nc.dram_tensor("out", [T, B, CSH], bf16, kind="ExternalOutput")

    with tile.TileContext(nc) as tc:
        with tc.tile_pool(name="sb", bufs=1) as sb:
            # ---- SBUF tiles -------------------------------------------------
            wv2_t = [
                sb.tile([128, 2 * C], bf16, name=f"wv{j}", tag=f"wv{j}")
                for j in range(KC // 2)
            ]
            warm_t = sb.tile([1, 512], bf16, tag="warm")
            wp_t = sb.tile([128, KC, CSH], bf16, tag="wp_t")
            vfeye_t = sb.tile([B, C + B], bf16, tag="vfeye")
            vft_t = sb.tile([128, KC * B], bf16, tag="vft")
            bvbp_t = sb.tile([1, C + CSH], bf16, tag="bvbp")
            sel_t = sb.tile([B, B * CSH], bf16, tag="sel")
            ones_t = sb.tile([1, B], bf16, tag="ones")
            ones_bc = sb.tile([B, 128], bf16, tag="ones_bc")
            vv_sb = sb.tile([B, C], bf16, tag="vv_sb")
            vvt_t = sb.tile([128, KC * B], bf16, tag="vvt")
            rhs4_t = sb.tile([B, B * CSH], bf16, tag="rhs4")
            bc_sb = sb.tile([128, B * CSH], bf16, tag="bc")

            nc.gpsimd.memset(ones_t[:], 1.0)
            nc.gpsimd.memset(ones_bc[:], 1.0)
            nc.gpsimd.memset(warm_t[:], 0.5)

            # ---- DMA in: both HWDGE queues; wv 2-chunk pairs (4KB lines) ----
            nc.sync.dma_start(vfeye_t[:], vfeye_b[:, :])
            nc.sync.dma_start(wv2_t[0][:], wv_b[:, 0 : 2 * C])
            nc.sync.dma_start(wv2_t[2][:], wv_b[:, 4 * C : 6 * C])
            nc.scalar.dma_start(bvbp_t[:], bvbp_b[:, :])
            nc.scalar.dma_start(wv2_t[1][:], wv_b[:, 2 * C : 4 * C])
            nc.scalar.dma_start(wv2_t[3][:], wv_b[:, 6 * C : 8 * C])
            # gpsimd/SWDGE (slow): only the late-needed tensors
            nc.gpsimd.dma_start(sel_t[:], sel_b[:, :])
            nc.gpsimd.dma_start(wp_t[:], wp_b.rearrange("p (k c) -> p k c", c=CSH))

            with (
                tc.tile_pool(name="pv", bufs=1, space="PSUM") as pv,
                tc.tile_pool(name="pf", bufs=1, space="PSUM") as pf,
                tc.tile_pool(name="pw", bufs=1, space="PSUM") as pw,
            ):
                psum_vv = [
                    pv.tile([B, 512], mybir.dt.float32, name=f"pvv{h}", tag=f"pvv{h}")
                    for h in range(2)
                ]
                # self-sufficient warm-up matmuls (memset inputs only -- no
                # DMA waits can be hoisted onto them): ramp the PE clock
                psum_warm = pw.tile([B, 512], mybir.dt.float32, tag="pwm")
                for _ in range(5):
                    nc.tensor.matmul(
                        psum_warm[:],
                        ones_t[0:1, :],
                        warm_t[0:1, :],
                        start=True,
                        stop=True,
                        skip_group_check=True,
                    )

                # ---- vf^T chunks via PE transpose (on the warming PE) -------
                psum_vft = pf.tile([128, KC * B], bf16, tag="pvf")
                for k in range(KC):
                    nc.tensor.transpose(
                        psum_vft[:, ts(k, B)],
                        vfeye_t[0:B, ts(k, 128)],
                        vfeye_t[0:B, C : C + B],
                    )
                nc.vector.tensor_copy(vft_t[:], psum_vft[:])

                # rank-1 bias terms: psum_vv[h] = 1^T bv_half
                for h in range(2):
                    nc.tensor.matmul(
                        psum_vv[h][:],
                        ones_t[0:1, :],
                        bvbp_t[0:1, ts(h, 512)],
                        start=True,
                        stop=False,
                        skip_group_check=True,
                    )

                # ---- mm1: vv[b, n] = bv[n] + sum_k vf[b, k] Wv[k, n] --------
                for k in range(KC):
                    for h in range(2):
                        nc.tensor.matmul(
                            psum_vv[h][:],
                            vft_t[:, ts(k, B)],
                            wv2_t[k // 2][:, ts(2 * (k % 2) + h, 512)],
                            start=False,
                            stop=(k == KC - 1),
                            skip_group_check=True,
                        )
                # psum -> SBUF bf16 cast, split across DVE + Act engines
                nc.vector.tensor_copy(vv_sb[0:B, 0:512], psum_vv[0][:])
                nc.scalar.activation(
                    vv_sb[0:B, 512:1024],
                    psum_vv[1][:],
                    mybir.ActivationFunctionType.Copy,
                )

            # ---- transpose vv -> vv^T into one psum tile, one copy out ------
            with tc.tile_pool(name="pt", bufs=1, space="PSUM") as pt:
                psum_vvt = pt.tile([128, KC * B], bf16, tag="pvt")
                for k in range(KC):
                    nc.tensor.transpose(
                        psum_vvt[:, ts(k, B)],
                        vv_sb[0:B, ts(k, 128)],
                        vfeye_t[0:B, C : C + B],
                    )
                nc.vector.tensor_copy(vvt_t[:], psum_vvt[:])

            # ---- mm2: row = bp + vv @ Wp[:,ci] ------------------------------
            with (
                tc.tile_pool(name="pr", bufs=1, space="PSUM") as pr,
                tc.tile_pool(name="pb", bufs=1, space="PSUM") as pb,
            ):
                psum_row = pr.tile([B, CSH], mybir.dt.float32, tag="pr")
                nc.tensor.matmul(
                    psum_row[:],
                    ones_t[0:1, :],
                    bvbp_t[0:1, C : C + CSH],
                    start=True,
                    stop=False,
                )
                for k in range(KC):
                    nc.tensor.matmul(
                        psum_row[:],
                        vvt_t[:, ts(k, B)],
                        wp_t[:, k, :],
                        start=False,
                        stop=(k == KC - 1),
                    )

                # ---- T-broadcast: bc[t, (q,c)] = row[q, c] ------------------
                pra = psum_row[:]
                prep = bass.AP(
                    pra.tensor, pra.offset, [list(pra.ap[0]), [0, B], list(pra.ap[1])]
                )
                nc.vector.tensor_mul(
                    rhs4_t[:].rearrange("p (q f) -> p q f", q=B),
                    prep,
                    sel_t[:].rearrange("p (q f) -> p q f", q=B),
                )
                psum_bc = pb.tile([128, B * CSH], mybir.dt.float32, tag="pb")
                nc.tensor.matmul(
                    psum_bc[:], ones_bc[0:B, :], rhs4_t[0:B, :], start=True, stop=True
                )
                # psum -> SBUF bf16 halves (DVE + Act in parallel), then the
                # replicated out-DMAs (step-0 over the 8 t-chunks)
                half = B * CSH // 2
                out_v = out.rearrange("(q p) b c -> p q (b c)", p=128)
                nc.vector.tensor_copy(bc_sb[:, 0:half], psum_bc[:, 0:half])
                nc.scalar.activation(
                    bc_sb[:, half:],
                    psum_bc[:, half:],
                    mybir.ActivationFunctionType.Copy,
                )
                for i, deng in ((0, nc.sync), (1, nc.scalar)):
                    ap = bc_sb[:, i * half : (i + 1) * half]
                    rep = bass.AP(
                        ap.tensor, ap.offset, [list(ap.ap[0]), [0, KC], list(ap.ap[1])]
                    )
                    deng.dma_start(out_v[:, :, i * half : (i + 1) * half], rep)

    nc.compile()
    return nc


def _get_built():
    global _BUILT
    if _BUILT is None:
        _BUILT = build_nc()
    return _BUILT


def make_in_maps(inputs):
    vf = np.asarray(inputs["visual_features"], np.float32)
    wv = np.asarray(inputs["Wv"], np.float32)
    wp = np.asarray(inputs["Wp"], np.float32)
    bv = np.asarray(inputs["bv"], np.float32)
    bp = np.asarray(inputs["bp"], np.float32)
    # wv_b[p, k*C + n] = Wv[k*128 + p, n]
    wv_b = np.ascontiguousarray(
        wv.reshape(KC, 128, C).transpose(1, 0, 2).reshape(128, KC * C)
    ).astype(BF16)
    vfeye_b = np.concatenate([vf, np.eye(B, dtype=np.float32)], axis=1).astype(BF16)
    sel_b = np.zeros((B, B * CSH), np.float32)
    for b in range(B):
        sel_b[b, b * CSH : (b + 1) * CSH] = 1.0
    sel_b = sel_b.astype(BF16)
    maps = []
    for i in range(N_CORES):
        ci = slice(i * CSH, (i + 1) * CSH)
        # wp_b[p, k*CSH + c] = Wp[k*128 + p, ci_c]
        wp_b = np.ascontiguousarray(
            wp[:, ci].reshape(KC, 128, CSH).transpose(1, 0, 2).reshape(128, KC * CSH)
        ).astype(BF16)
        bvbp_b = np.concatenate([bv, bp[ci]]).reshape(1, C + CSH).astype(BF16)
        maps.append(
            {
                "wv_b": wv_b,
                "wp_b": wp_b,
                "vfeye_b": vfeye_b,
                "bvbp_b": bvbp_b,
                "sel_b": sel_b,
            }
        )
    return maps


def run(inputs, trace=False, **kw):
    from concourse.bass_utils import run_bass_kernel_spmd

    nc = _get_built()
    res = run_bass_kernel_spmd(
        nc,
        make_in_maps(inputs),
        core_ids=list(range(N_CORES)),
        trace=trace,
        **kw,
    )
    full = np.empty((B, T, C), np.float32)
    for i, r in enumerate(res.results):
        full[:, :, i * CSH : (i + 1) * CSH] = (
            r["out"].astype(np.float32).transpose(1, 0, 2)
        )
    return full, res


def kernel(**inputs) -> np.ndarray:
    full, _ = run(inputs, trace=False)
    return full
